# revision 1
# baseline (speedup 1.0000x reference)
"""Deformable conv Bass kernel for TRN2, 8-core SPMD.

Sharding: core = b*2 + h  (b = batch 0..3, h = row-half 0..1).
Each core computes out[b, :, 32h:32h+32, :]  (2048 output pixels).

Partition-row layout for om/coords (free dim = 1024 j within row-half phi2):
  x-offset rows:  k*2 + phi2        (k in 0..8)  -> rows 0..17
  y-offset rows:  32 + k*2 + phi2                -> rows 32..49
  mask rows:      64 + k*2 + phi2                -> rows 64..81
Gather slot order: slot = (k*2+phi2)*1024 + j = k*2048 + p  (p = pixel in half).

Token table (per batch): token(yc, xc) = [x[:, y0, x0], x[:, y0+1, x0]] bf16,
y0 = yc-2, x0 = xc-2, zeros where out of bounds. idx = yc*68 + xc.
Left corner token idx = (y0c+2)*68 + (x0c+2); right = idx+1.
"""
import sys
sys.path.insert(0, '/opt/trn_rl_repo')
from contextlib import ExitStack

import numpy as np
import ml_dtypes

import concourse.bass as bass
import concourse.bacc as bacc
import concourse.tile as tile
from concourse import mybir

F32 = mybir.dt.float32
BF16 = mybir.dt.bfloat16
I16 = mybir.dt.int16
AOP = mybir.AluOpType
ACTF = mybir.ActivationFunctionType

K = 3
KK = 9
B, CIN, COUT, H, W = 4, 128, 256, 64, 64
N_CORES = 8
HHALF = 32               # output rows per core
NPIX = HHALF * W         # 2048
NJ = 1024                # free dim per phi2 half
NROWS = 34               # xpad slab rows per core
TABW = 68                # token table side (y0,x0 in -2..65)
NTOK = TABW * TABW       # 4624
ESZ = 2 * CIN            # token elems (bf16) = 256 -> 512B
SCALE = np.float32(W / (W - 1.0))
MAGIC = 12582912.0       # 1.5 * 2^23 fp32 round-to-int trick


def _taps():
    r = [-1, 0, 1]
    return [(r[k // 3], r[k % 3]) for k in range(KK)]  # (koy, kox)


# ---------------------------------------------------------------- host prep
def host_prep(x, om_w, om_b, weight, bias):
    """Build all per-core input dicts. Returns list of 8 dicts."""
    x = np.asarray(x, np.float32)
    om_w = np.asarray(om_w, np.float32)
    om_b = np.asarray(om_b, np.float32)
    weight = np.asarray(weight, np.float32)
    bias = np.asarray(bias, np.float32)
    taps = _taps()

    # padded image per batch [128, 66, 66]
    xpad = np.zeros((B, CIN, H + 2, W + 2), np.float32)
    xpad[:, :, 1:-1, 1:-1] = x

    # om conv lhsT variants [9 taps, 2 halves, 128 c, 96 M]
    omw_l = np.zeros((KK, 2, CIN, 96), np.float32)
    for t in range(KK):
        ty, tx = t // 3, t % 3
        for phi2 in range(2):
            for k in range(KK):
                m = k * 2 + phi2
                omw_l[t, phi2, :, m] = om_w[k, :, ty, tx]               # off_x
                omw_l[t, phi2, :, 32 + m] = om_w[KK + k, :, ty, tx]     # off_y
                omw_l[t, phi2, :, 64 + m] = om_w[2 * KK + k, :, ty, tx]  # mask

    # token tables per batch [NTOK, 256] bf16
    ztab = np.zeros((B, NTOK, ESZ), ml_dtypes.bfloat16)
    xb = x.astype(ml_dtypes.bfloat16)
    for yc in range(TABW):
        y0 = yc - 2
        for rr in range(2):
            yy = y0 + rr
            if 0 <= yy < H:
                ztab[:, yc * TABW + 2: yc * TABW + 2 + W, rr * CIN:(rr + 1) * CIN] = \
                    xb[:, :, yy, :].transpose(0, 2, 1)

    # main weights [9, 128, 256] bf16
    wmain = np.empty((KK, CIN, COUT), ml_dtypes.bfloat16)
    for k in range(KK):
        wmain[k] = weight[:, :, k // 3, k % 3].T.astype(ml_dtypes.bfloat16)

    biasv = np.ascontiguousarray(bias.reshape(2, 128).T.astype(np.float32))  # [128,2]

    svec = np.zeros((128, 1), np.float32)
    svec[0:18] = SCALE
    svec[32:50] = SCALE
    svec[64:82] = 1.0
    lov = np.zeros((64, 1), np.float32)
    hiv = np.zeros((64, 1), np.float32)
    lov[0:18] = -2.0; hiv[0:18] = 64.0    # x0 clip
    lov[32:50] = -2.0; hiv[32:50] = 65.0  # y0 clip

    Bpl = np.zeros((2, 128, NJ), np.float32)  # per h
    jj = np.arange(NJ)
    jrow, jx = jj // W, jj % W
    for h in range(2):
        for phi2 in range(2):
            for k in range(KK):
                koy, kox = taps[k]
                m = k * 2 + phi2
                bx = jx - 1 + kox
                y = 32 * h + 16 * phi2 + jrow
                by = y - 1 + koy
                # i1 = scale*om + B ; want i1 = ix - 0.5 = scale*(bx+off+om_b) - 1
                Bpl[h, m] = SCALE * (bx + om_b[k]) - 1.0
                Bpl[h, 32 + m] = SCALE * (by + om_b[KK + k]) - 1.0
                Bpl[h, 64 + m] = om_b[2 * KK + k]

    ins = []
    for core in range(N_CORES):
        b, h = core // 2, core % 2
        ins.append({
            "xpad": np.ascontiguousarray(
                xpad[b, :, 32 * h:32 * h + NROWS, :].reshape(CIN, NROWS * 66)),
            "omw": omw_l,
            "Bpl": Bpl[h],
            "svec": svec,
            "lov": lov,
            "hiv": hiv,
            "ztab": np.ascontiguousarray(ztab[b]),
            "wmain": wmain,
            "biasv": biasv,
        })
    return ins


def assemble(results):
    """results: list of 8 dicts with 'out' [2, 128, NPIX] -> [B, COUT, H, W]"""
    out = np.empty((B, COUT, H, W), np.float32)
    for core in range(N_CORES):
        b, h = core // 2, core % 2
        o = np.asarray(results[core]["out"], np.float32).reshape(COUT, HHALF, W)
        out[b, :, 32 * h:32 * h + HHALF, :] = o
    return out


# ---------------------------------------------------------------- kernel
def build_kernel():
    nc = bacc.Bacc("TRN2", target_bir_lowering=False, debug=False,
                   enable_asserts=True, num_devices=N_CORES)
    d = {}
    d["xpad"] = nc.dram_tensor("xpad", [CIN, NROWS * 66], F32, kind="ExternalInput").ap()
    d["omw"] = nc.dram_tensor("omw", [KK, 2, CIN, 96], F32, kind="ExternalInput").ap()
    d["Bpl"] = nc.dram_tensor("Bpl", [128, NJ], F32, kind="ExternalInput").ap()
    d["svec"] = nc.dram_tensor("svec", [128, 1], F32, kind="ExternalInput").ap()
    d["lov"] = nc.dram_tensor("lov", [64, 1], F32, kind="ExternalInput").ap()
    d["hiv"] = nc.dram_tensor("hiv", [64, 1], F32, kind="ExternalInput").ap()
    d["ztab"] = nc.dram_tensor("ztab", [NTOK, ESZ], BF16, kind="ExternalInput").ap()
    d["wmain"] = nc.dram_tensor("wmain", [KK, CIN, COUT], BF16, kind="ExternalInput").ap()
    d["biasv"] = nc.dram_tensor("biasv", [128, 2], F32, kind="ExternalInput").ap()
    d["out"] = nc.dram_tensor("out", [2, 128, NPIX], F32, kind="ExternalOutput").ap()

    with tile.TileContext(nc) as tc:
        _build(nc, tc, d)
    nc.compile()
    return nc


def _build(nc, tc, d):
    ctx = ExitStack()
    const = ctx.enter_context(tc.tile_pool(name="const", bufs=1))
    work = ctx.enter_context(tc.tile_pool(name="work", bufs=1))
    gpool = ctx.enter_context(tc.tile_pool(name="gpool", bufs=2))
    pspool = ctx.enter_context(tc.tile_pool(name="psum", bufs=2, space="PSUM"))
    dpool = ctx.enter_context(tc.tile_pool(name="dram", bufs=1, space="DRAM"))

    # ---- load constants
    xpad_t = const.tile([CIN, NROWS * 66], F32)
    nc.sync.dma_start(out=xpad_t, in_=d["xpad"])
    omw_t = const.tile([CIN, KK, 2, 96], F32)
    for t in range(KK):
        nc.sync.dma_start(
            out=omw_t[:, t, :, :],
            in_=bass.AP(tensor=d["omw"].tensor, offset=t * 2 * CIN * 96,
                        ap=[[96, CIN], [CIN * 96, 2], [1, 96]]))
    B_t = const.tile([128, NJ], F32)
    nc.sync.dma_start(out=B_t, in_=d["Bpl"])
    sv = const.tile([128, 1], F32)
    nc.sync.dma_start(out=sv, in_=d["svec"])
    lov = const.tile([64, 1], F32)
    nc.sync.dma_start(out=lov, in_=d["lov"])
    hiv = const.tile([64, 1], F32)
    nc.sync.dma_start(out=hiv, in_=d["hiv"])
    wm_t = const.tile([CIN, KK, COUT], BF16)
    for k in range(KK):
        nc.sync.dma_start(
            out=wm_t[:, k, :],
            in_=bass.AP(tensor=d["wmain"].tensor, offset=k * CIN * COUT,
                        ap=[[COUT, CIN], [1, COUT]]))
    bias_t = const.tile([128, 2], F32)
    nc.sync.dma_start(out=bias_t, in_=d["biasv"])

    # ---- om conv: 9 taps x 2 halves x 2 n-chunks -> psum [82, 2, 512]
    om_ps = pspool.tile([128, 4, 512], F32, tag="ps")
    xv = xpad_t[:, :].rearrange("c (yy xx) -> c yy xx", yy=NROWS)
    for t in range(KK):
        ty, tx = t // 3, t % 3
        for phi2 in range(2):
            for n2 in range(2):
                r0 = 16 * phi2 + 8 * n2 + ty
                rhs = xv[:, r0:r0 + 8, tx:tx + W]   # [128, 8, 64] = 512 free
                nc.tensor.matmul(
                    om_ps[0:82, n2, :],
                    omw_t[:, t, phi2, 0:82],
                    rhs,
                    start=(t == 0 and phi2 == 0),
                    stop=(t == KK - 1 and phi2 == 1),
                )
    omv = om_ps[0:82, 0:2, :].rearrange("p a b -> p (a b)")  # [82, 1024]

    # ---- coords
    i1 = work.tile([82, NJ], F32)
    nc.vector.scalar_tensor_tensor(out=i1, in0=omv, scalar=sv[0:82],
                                   in1=B_t[0:82], op0=AOP.mult, op1=AOP.add)
    # mask = sigmoid(logits rows 64..81)
    msk = work.tile([18, NJ], F32)
    nc.scalar.activation(out=msk, in_=i1[64:82], func=ACTF.Sigmoid)
    # floor(i1 + 0.5) = rne(i1) via magic; i1 = ix - 0.5 so f = floor(ix)
    f = work.tile([64, NJ], F32)
    nc.vector.tensor_scalar_add(f, i1[0:64], MAGIC)
    nc.vector.tensor_scalar_sub(f, f, MAGIC)
    # frac w1 = (i1 + 0.5) - f
    w1 = work.tile([64, NJ], F32)
    nc.vector.tensor_tensor(w1, i1[0:64], f, AOP.subtract)
    nc.vector.tensor_scalar_add(w1, w1, 0.5)
    # clip f
    nc.vector.tensor_scalar_max(f, f, lov)
    nc.vector.tensor_scalar_min(f, f, hiv)
    # copy y-rows down to base partition 0 (verifier: multi-input DVE ops
    # need equal SBUF base partitions)
    fy = work.tile([18, NJ], F32)
    nc.vector.tensor_copy(out=fy, in_=f[32:50])
    wy1c = work.tile([18, NJ], F32)
    nc.vector.tensor_copy(out=wy1c, in_=w1[32:50])
    # il = (y0c*68 + x0c) + (2*68 + 2) ; ir = il + 1
    il = work.tile([18, NJ], F32)
    nc.vector.scalar_tensor_tensor(out=il, in0=fy, scalar=float(TABW),
                                   in1=f[0:18], op0=AOP.mult, op1=AOP.add)
    nc.vector.tensor_scalar_add(il, il, float(2 * TABW + 2))
    ir = work.tile([18, NJ], F32)
    nc.vector.tensor_scalar_add(ir, il, 1.0)
    i16l = work.tile([18, NJ], I16)
    nc.vector.tensor_copy(out=i16l, in_=il)
    i16r = work.tile([18, NJ], I16)
    nc.vector.tensor_copy(out=i16r, in_=ir)

    # corner coefficient planes -> bf16 [18, 4, NJ]: jj = 0:cw00 1:cw01 2:cw10 3:cw11
    wy0 = work.tile([18, NJ], F32)
    nc.vector.tensor_scalar(out=wy0, in0=wy1c, scalar1=-1.0, scalar2=1.0,
                            op0=AOP.mult, op1=AOP.add)
    wxm1 = work.tile([18, NJ], F32)
    nc.vector.tensor_tensor(wxm1, w1[0:18], msk, AOP.mult)
    wxm0 = work.tile([18, NJ], F32)
    nc.vector.tensor_tensor(wxm0, msk, wxm1, AOP.subtract)
    cwb = work.tile([18, 4, NJ], BF16)
    nc.vector.tensor_tensor(cwb[:, 0, :], wy0, wxm0, AOP.mult)
    nc.vector.tensor_tensor(cwb[:, 1, :], wy1c, wxm0, AOP.mult)
    nc.vector.tensor_tensor(cwb[:, 2, :], wy0, wxm1, AOP.mult)
    nc.vector.tensor_tensor(cwb[:, 3, :], wy1c, wxm1, AOP.mult)

    # ---- DRAM round trips: idx wrap + coefficient planes
    idxl_dr = dpool.tile([18, NJ], I16)
    idxr_dr = dpool.tile([18, NJ], I16)
    cw_dr = dpool.tile([18, 4, NJ], BF16)
    wr_l = nc.sync.dma_start(out=idxl_dr, in_=i16l)
    wr_r = nc.sync.dma_start(out=idxr_dr, in_=i16r)
    wr_cw = nc.sync.dma_start(out=cw_dr, in_=cwb)

    NIDX = KK * 2 * NJ  # 18432
    iwl = const.tile([128, NIDX // 16], I16)
    iwr = const.tile([128, NIDX // 16], I16)
    for g in range(8):
        rd = nc.sync.dma_start(
            out=iwl[16 * g:16 * (g + 1), :],
            in_=bass.AP(tensor=idxl_dr.tensor, offset=idxl_dr[:].offset,
                        ap=[[1, 16], [16, NIDX // 16]]))
        tile.add_dep_helper(rd.ins, wr_l.ins, sync=True, reason="dram idx rt")
        rd = nc.sync.dma_start(
            out=iwr[16 * g:16 * (g + 1), :],
            in_=bass.AP(tensor=idxr_dr.tensor, offset=idxr_dr[:].offset,
                        ap=[[1, 16], [16, NIDX // 16]]))
        tile.add_dep_helper(rd.ins, wr_r.ins, sync=True, reason="dram idx rt")

    # ---- per-tap: gather, bilinear reduce, matmul accumulate
    main_ps = []
    for m in range(2):
        mps = pspool.tile([128, 4, 512], F32, tag="ps", name=f"main_ps{m}")
        main_ps.append(mps)
    GC = 512
    NCH = NPIX // GC              # 8 chunks per tap
    for k in range(KK):
        # chunked layout [128, chunk, rowpair, GC]; flat free order = pixel order
        gl = gpool.tile([128, NCH, 2, GC], BF16, tag="gl")
        gr = gpool.tile([128, NCH, 2, GC], BF16, tag="gr")
        SW = GC // 16
        for c8 in range(NCH):
            s0 = 128 * k + SW * c8
            nc.gpsimd.dma_gather(out_ap=gl[:, c8, :, :], in_ap=d["ztab"],
                                 idxs_ap=iwl[:, s0:s0 + SW],
                                 num_idxs=GC, num_idxs_reg=GC,
                                 elem_size=ESZ, transpose=True)
            nc.gpsimd.dma_gather(out_ap=gr[:, c8, :, :], in_ap=d["ztab"],
                                 idxs_ap=iwr[:, s0:s0 + SW],
                                 num_idxs=GC, num_idxs_reg=GC,
                                 elem_size=ESZ, transpose=True)
        cwbl = gpool.tile([128, NCH, 2, GC], BF16, tag="cwbl")
        cwbr = gpool.tile([128, NCH, 2, GC], BF16, tag="cwbr")
        cwbase = cw_dr[:].offset
        for phi2 in range(2):
            for c4 in range(NJ // GC):
                ch = phi2 * (NJ // GC) + c4
                rd = nc.sync.dma_start(
                    out=cwbl[:, ch, :, :],
                    in_=bass.AP(tensor=cw_dr.tensor,
                                offset=cwbase + ((k * 2 + phi2) * 4 + 0) * NJ + c4 * GC,
                                ap=[[0, 128], [NJ, 2], [1, GC]]))
                tile.add_dep_helper(rd.ins, wr_cw.ins, sync=True, reason="dram cw rt")
                rd = nc.sync.dma_start(
                    out=cwbr[:, ch, :, :],
                    in_=bass.AP(tensor=cw_dr.tensor,
                                offset=cwbase + ((k * 2 + phi2) * 4 + 2) * NJ + c4 * GC,
                                ap=[[0, 128], [NJ, 2], [1, GC]]))
                tile.add_dep_helper(rd.ins, wr_cw.ins, sync=True, reason="dram cw rt")
        # products (in place over gl/gr) then sum 4 corners
        nc.vector.tensor_tensor(gl[:], gl[:], cwbl[:], AOP.mult)
        nc.vector.tensor_tensor(gr[:], gr[:], cwbr[:], AOP.mult)
        samp = gpool.tile([128, NPIX], BF16, tag="samp")
        sB = gpool.tile([128, NPIX], BF16, tag="sB")
        sampv = samp[:, :].rearrange("c (a b) -> c a b", a=NCH)
        sBv = sB[:, :].rearrange("c (a b) -> c a b", a=NCH)
        nc.vector.tensor_tensor(sampv, gl[:, :, 0, :], gl[:, :, 1, :], AOP.add)
        nc.vector.tensor_tensor(sBv, gr[:, :, 0, :], gr[:, :, 1, :], AOP.add)
        nc.vector.tensor_tensor(samp, samp, sB, AOP.add)
        for m in range(2):
            for n4 in range(4):
                nc.tensor.matmul(
                    main_ps[m][:, n4, :],
                    wm_t[:, k, 128 * m:128 * (m + 1)],
                    samp[:, 512 * n4:512 * (n4 + 1)],
                    start=(k == 0),
                    stop=(k == KK - 1),
                )

    # ---- bias + copy out
    for m in range(2):
        osb = gpool.tile([128, NPIX], F32, tag="osb")
        for n4 in range(4):
            nc.scalar.activation(out=osb[:, 512 * n4:512 * (n4 + 1)],
                                 in_=main_ps[m][:, n4, :],
                                 func=ACTF.Identity,
                                 bias=bias_t[:, m:m + 1], scale=1.0)
        nc.sync.dma_start(out=d["out"][m], in_=osb)
    ctx.close()


# ---------------------------------------------------------------- entry point
_NC_CACHE = []


def kernel(x, om_w, om_b, weight, bias):
    """Full-input deformable conv on 8 NeuronCores; returns [4,256,64,64] f32."""
    from concourse import bass_utils
    if not _NC_CACHE:
        _NC_CACHE.append(build_kernel())
    nc = _NC_CACHE[0]
    ins = host_prep(x, om_w, om_b, weight, bias)
    res = bass_utils.run_bass_kernel_spmd(nc, ins, core_ids=list(range(N_CORES)))
    out = assemble(res.results)
    return out.astype(np.float32)

